# revision 8
# baseline (speedup 1.0000x reference)
"""APPNP GNN kernel for 8 TRN2 NeuronCores — NG=8 / d=2 gather layout.

Key change vs baseline: z is packed 4 channels per lane (2 u32 per node per
lane), 8 partition groups of 16 lanes (one Q7 core each). The main gather
uses d=2, halving the index count for the same data volume — ap_gather cost
is ~28ns/index regardless of d, so the gather (the dominant cost) halves.

Layout:
- zpk [128, npcp, 2] u32: row 16g+l holds nodes of block g (12500 nodes,
  padded to npcp=12504); bf16 position j in the 2-u32 block = channel 4l+j.
- Per-core outputs znew/h_a use the same 16-lane pattern with node OCTANTS:
  row 16o+l, col (n, j) = node oct*o+n, channel 4l+j (oct=1563).
- Per iteration, per range r: ap_gather(d=2) packed z by src; TensorE
  broadcasts per-edge weights [8 -> 128 lanes] into PSUM; DVE mult
  (4 planes, strided read); DVE inclusive-scan per plane (fp32, interleaved
  x4 write); ap_gather(d=4) segment-end prefix values; TensorE merges the
  8 group partials (one [128,128] matmul, rows folded mod 16); DVE diff +
  alpha*h epilogue into znew. AllGather redistributes z (bf16).
"""

import numpy as np
import ml_dtypes

import concourse.bass as bass
import concourse.bacc as bacc
import concourse.mybir as mybir
import concourse.tile as tile
from concourse.bass_utils import run_bass_kernel_spmd

dt = mybir.dt
AOP = mybir.AluOpType
ALPHA = 0.1
M = 8   # cores
NG = 8  # src blocks / partition groups (16 lanes each)


# ---------------------------------------------------------------- host prep
def build_plan(n_nodes, edge_src, edge_dst, edge_weight, cap=991, span_cap=252):
    """Per-core edge plan with GLOBAL (SPMD-identical) range structure."""
    npc = n_nodes // M          # 12500
    oct_ = (npc + NG - 1) // NG  # 1563
    oct_ += oct_ % 2             # keep even (u32 pair alignment safety)
    blk = npc                    # src block size == nodes per core
    owner = edge_dst // npc
    cores = []
    for c in range(M):
        sel = np.nonzero(owner == c)[0]
        src = edge_src[sel].astype(np.int64)
        dstl = (edge_dst[sel] - c * npc).astype(np.int64)
        w = (edge_weight[sel] * (1.0 - ALPHA)).astype(np.float32)
        g = src // blk
        srcl = (src % blk).astype(np.int32)
        streams = []
        cums = []
        for gi in range(NG):
            gsel = np.nonzero(g == gi)[0]
            order = np.argsort(dstl[gsel], kind="stable")
            ge = gsel[order]
            cnt = np.bincount(dstl[gsel], minlength=npc)
            streams.append((srcl[ge], w[ge]))
            cums.append(np.concatenate([[0], np.cumsum(cnt)]))
        cores.append((streams, cums))

    # global range breakpoints (shared by all cores); never cross octants
    allcums = [cu for _, cums in cores for cu in cums]
    cuts = [0]
    n0 = 0
    while n0 < npc:
        hi = min(n0 + span_cap, npc, (n0 // oct_ + 1) * oct_)
        n1 = hi
        while n1 > n0 + 1:
            if all(cu[n1] - cu[n0] <= cap for cu in allcums):
                break
            n1 -= 1
        cuts.append(n1)
        n0 = n1
    ranges = []
    for i in range(len(cuts) - 1):
        n0, n1 = cuts[i], cuts[i + 1]
        nr = n1 - n0
        p = 1 + max(cu[n1] - cu[n0] for cu in allcums)
        p = (p + 31) // 32 * 32  # p/16 even keeps idx slices 4B-aligned
        nbi = (nr + 3) // 4 * 4
        nbi_cols = (nbi + 15) // 16
        nbi_cols += nbi_cols % 2
        ranges.append(dict(n0=n0, n1=n1, nr=nr, p=p, nbi=nbi, nbi_cols=nbi_cols))
    io = bo = wo = 0
    for r in ranges:
        r["io"] = io
        r["bo"] = bo
        r["wo"] = wo
        io += r["p"] // 16
        bo += r["nbi_cols"]
        wo += r["p"]
    idx_cols, bidx_cols, w_cols = io, bo, wo

    plans = []
    for c in range(M):
        streams, cums = cores[c]
        idx_sb = np.zeros((128, idx_cols), dtype=np.int16)
        bidx_sb = np.full((128, bidx_cols), -1, dtype=np.int16)
        w_flat = np.zeros((NG, w_cols), dtype=np.float32)
        for r in ranges:
            p, n0, n1, nr = r["p"], r["n0"], r["n1"], r["nr"]
            io, bo, wo = r["io"], r["bo"], r["wo"]
            for gi in range(NG):
                sl, wl = streams[gi]
                a, b = cums[gi][n0], cums[gi][n1]
                ids = np.zeros(p, dtype=np.int16)
                ids[1 : 1 + (b - a)] = sl[a:b]
                idx_wrap = ids.reshape(p // 16, 16).T
                rr = 16 * gi
                idx_sb[rr : rr + 16, io : io + p // 16] = idx_wrap
                w_flat[gi, wo] = 0.0
                w_flat[gi, wo + 1 : wo + 1 + (b - a)] = wl[a:b]
                bpos = (cums[gi][n0 + 1 : n1 + 1] - a).astype(np.int16)
                bb = np.full(r["nbi_cols"] * 16, -1, dtype=np.int16)
                bb[:nr] = bpos
                bwrap = bb.reshape(r["nbi_cols"], 16).T
                bidx_sb[rr : rr + 16, bo : bo + r["nbi_cols"]] = bwrap
        plans.append(dict(ranges=ranges, idx_sb=idx_sb, bidx_sb=bidx_sb,
                          w_flat=w_flat, oct=oct_))
    return plans


# W3 column permutation: h3 row 16j+l holds channel 4l+j
PERM = np.zeros(64, dtype=np.int64)
for _j in range(4):
    for _l in range(16):
        PERM[16 * _j + _l] = 4 * _l + _j

SEL8 = np.zeros((NG, 128), dtype=np.float32)
for _g in range(NG):
    SEL8[_g, 16 * _g : 16 * _g + 16] = 1.0

# merge: out[16o+l] = sum_g in[16g+l]  (same value for every octant row)
MERGE8 = np.zeros((128, 128), dtype=np.float32)
for _r in range(128):
    for _c in range(128):
        if _r % 16 == _c % 16:
            MERGE8[_r, _c] = 1.0


def host_inputs(x, W1, b1, W2, b2, W3, b3, plans, n_nodes):
    npc = n_nodes // M
    bf16 = ml_dtypes.bfloat16
    in_maps = []
    xT = np.ascontiguousarray(x.T.astype(bf16))  # [512, N]
    W3p = np.ascontiguousarray(W3[:, PERM])
    b3p = np.ascontiguousarray(b3[PERM])
    for c in range(M):
        pl = plans[c]
        in_maps.append(
            {
                "xT": np.ascontiguousarray(xT[:, c * npc : (c + 1) * npc]),
                "W1": W1.astype(bf16),
                "W2": W2.astype(bf16),
                "W3": W3p.astype(bf16),
                "b1": b1.astype(np.float32),
                "b2": b2.astype(np.float32),
                "b3": b3p.astype(np.float32),
                "idx": pl["idx_sb"],
                "bidx": pl["bidx_sb"],
                "wf": pl["w_flat"],
                "sel8": SEL8,
                "merge": MERGE8,
            }
        )
    return in_maps


# ---------------------------------------------------------------- builder
def build_kernel(n_nodes, in_ch, hid_ch, out_ch, k_iters, plans):
    npc = n_nodes // M
    oct_ = plans[0]["oct"]
    npcp = oct_ * NG
    assert out_ch == 64
    nc = bacc.Bacc("TRN2", target_bir_lowering=False, num_devices=M)

    pl0 = plans[0]
    idx_cols = pl0["idx_sb"].shape[1]
    bidx_cols = pl0["bidx_sb"].shape[1]
    w_cols = pl0["w_flat"].shape[1]
    ranges = pl0["ranges"]
    for pl in plans[1:]:
        assert pl["idx_sb"].shape == pl0["idx_sb"].shape
        assert pl["w_flat"].shape == pl0["w_flat"].shape
        assert [r["p"] for r in pl["ranges"]] == [r["p"] for r in ranges]
        assert [r["n0"] for r in pl["ranges"]] == [r["n0"] for r in ranges]

    PMAX = max(r["p"] for r in ranges)
    NBIMAX = max(r["nbi"] for r in ranges)
    assert PMAX <= 1024 and NBIMAX <= 256, (PMAX, NBIMAX)

    # ---- dram I/O
    d_xT = nc.dram_tensor("xT", [in_ch, npc], dt.bfloat16, kind="ExternalInput")
    d_W1 = nc.dram_tensor("W1", [in_ch, hid_ch], dt.bfloat16, kind="ExternalInput")
    d_W2 = nc.dram_tensor("W2", [hid_ch, hid_ch], dt.bfloat16, kind="ExternalInput")
    d_W3 = nc.dram_tensor("W3", [hid_ch, out_ch], dt.bfloat16, kind="ExternalInput")
    d_b1 = nc.dram_tensor("b1", [hid_ch], dt.float32, kind="ExternalInput")
    d_b2 = nc.dram_tensor("b2", [hid_ch], dt.float32, kind="ExternalInput")
    d_b3 = nc.dram_tensor("b3", [out_ch], dt.float32, kind="ExternalInput")
    d_idx = nc.dram_tensor("idx", [128, idx_cols], dt.int16, kind="ExternalInput")
    d_bidx = nc.dram_tensor("bidx", [128, bidx_cols], dt.int16, kind="ExternalInput")
    d_wf = nc.dram_tensor("wf", [NG, w_cols], dt.float32, kind="ExternalInput")
    d_sel8 = nc.dram_tensor("sel8", [NG, 128], dt.float32, kind="ExternalInput")
    d_merge = nc.dram_tensor("merge", [128, 128], dt.float32, kind="ExternalInput")
    d_zout = nc.dram_tensor("zout", [128, oct_, 4], dt.float32, kind="ExternalOutput")

    d_hb = nc.dram_tensor("d_hb", [64, npc], dt.bfloat16)
    d_hab = nc.dram_tensor("d_hab", [64, npc], dt.bfloat16)
    cc_in = nc.dram_tensor("cc_in", [16, 4 * npcp], dt.bfloat16)
    cc_out = nc.dram_tensor("cc_out", [128, 4 * npcp], dt.bfloat16, addr_space="Shared")

    with tile.TileContext(nc) as tc:
        with (
            tc.tile_pool(name="psum", bufs=1, space="PSUM") as ppool,
        ):
            # ---------------- MLP: h_T = mlp(x) computed transposed
            NCH = 500 if npc % 500 == 0 else 256
            assert npc % NCH == 0
            nchunks = npc // NCH
            with (
                tc.tile_pool(name="mlp", bufs=1) as mp,
                tc.tile_pool(name="mlp_ps", bufs=4, space="PSUM") as mpp,
            ):
                w1t = [mp.tile([128, hid_ch], dt.bfloat16, tag=f"w1_{i}", name=f"w1_{i}") for i in range(in_ch // 128)]
                w2t = [mp.tile([128, hid_ch], dt.bfloat16, tag=f"w2_{i}", name=f"w2_{i}") for i in range(hid_ch // 128)]
                w3t = [mp.tile([128, out_ch], dt.bfloat16, tag=f"w3_{i}", name=f"w3_{i}") for i in range(hid_ch // 128)]
                for i, t in enumerate(w1t):
                    nc.sync.dma_start(out=t[:], in_=d_W1[128 * i : 128 * (i + 1), :])
                for i, t in enumerate(w2t):
                    nc.sync.dma_start(out=t[:], in_=d_W2[128 * i : 128 * (i + 1), :])
                for i, t in enumerate(w3t):
                    nc.sync.dma_start(out=t[:], in_=d_W3[128 * i : 128 * (i + 1), :])
                b1t = mp.tile([128, 2], dt.float32, tag="bias")
                b2t = mp.tile([128, 2], dt.float32, tag="bias2")
                b3t = mp.tile([64, 1], dt.float32, tag="bias3")
                for h2 in range(2):
                    nc.sync.dma_start(out=b1t[:, h2 : h2 + 1], in_=d_b1[128 * h2 : 128 * (h2 + 1), None])
                    nc.sync.dma_start(out=b2t[:, h2 : h2 + 1], in_=d_b2[128 * h2 : 128 * (h2 + 1), None])
                nc.sync.dma_start(out=b3t[:], in_=d_b3[:, None])

                xt = [mp.tile([128, npc], dt.bfloat16, tag=f"big{i}", name=f"xt_{i}", bufs=1) for i in range(in_ch // 128)]
                for i, t in enumerate(xt):
                    nc.sync.dma_start(out=t[:], in_=d_xT[128 * i : 128 * (i + 1), :])
                h1 = [mp.tile([128, npc], dt.bfloat16, tag=f"h1_{i}", name=f"h1_{i}") for i in range(hid_ch // 128)]
                for m in range(hid_ch // 128):
                    for j in range(nchunks):
                        ps = mpp.tile([128, NCH], dt.float32, space="PSUM", tag="ps")
                        for k in range(in_ch // 128):
                            nc.tensor.matmul(
                                out=ps[:],
                                lhsT=w1t[k][:, 128 * m : 128 * (m + 1)],
                                rhs=xt[k][:, j * NCH : (j + 1) * NCH],
                                start=(k == 0),
                                stop=(k == in_ch // 128 - 1),
                            )
                        nc.scalar.activation(
                            out=h1[m][:, j * NCH : (j + 1) * NCH],
                            in_=ps[:],
                            func=mybir.ActivationFunctionType.Relu,
                            bias=b1t[:, m : m + 1],
                            scale=1.0,
                        )
                h2t = [mp.tile([128, npc], dt.bfloat16, tag=f"big{i}", name=f"h2_{i}") for i in range(hid_ch // 128)]
                for m in range(hid_ch // 128):
                    for j in range(nchunks):
                        ps = mpp.tile([128, NCH], dt.float32, space="PSUM", tag="ps")
                        for k in range(hid_ch // 128):
                            nc.tensor.matmul(
                                out=ps[:],
                                lhsT=w2t[k][:, 128 * m : 128 * (m + 1)],
                                rhs=h1[k][:, j * NCH : (j + 1) * NCH],
                                start=(k == 0),
                                stop=(k == hid_ch // 128 - 1),
                            )
                        nc.scalar.activation(
                            out=h2t[m][:, j * NCH : (j + 1) * NCH],
                            in_=ps[:],
                            func=mybir.ActivationFunctionType.Relu,
                            bias=b2t[:, m : m + 1],
                            scale=1.0,
                        )
                # h3: [64, npc] fp32 (rows already channel-permuted via W3p)
                hsplit = (npc // 2) // NCH * NCH
                h3_sizes = [hsplit, npc - hsplit]
                h3 = [mp.tile([64, h3_sizes[i]], dt.float32, tag=f"big{i+2}", name=f"h3_{i}") for i in range(2)]
                for j in range(nchunks):
                    ps = mpp.tile([64, NCH], dt.float32, space="PSUM", tag="ps3")
                    for k in range(hid_ch // 128):
                        nc.tensor.matmul(
                            out=ps[:],
                            lhsT=w3t[k][:],
                            rhs=h2t[k][:, j * NCH : (j + 1) * NCH],
                            start=(k == 0),
                            stop=(k == hid_ch // 128 - 1),
                        )
                    half = 0 if (j * NCH) < hsplit else 1
                    off = j * NCH - half * hsplit
                    nc.vector.tensor_scalar_add(
                        out=h3[half][:, off : off + NCH],
                        in0=ps[:],
                        scalar1=b3t[:],
                    )

                # cast h3 -> bf16 (and alpha-scaled), bounce to DRAM
                h3b = [mp.tile([64, h3_sizes[i]], dt.bfloat16, tag=f"h1_{i}", name=f"h3b_{i}") for i in range(2)]
                h3ab = [mp.tile([64, h3_sizes[i]], dt.bfloat16, tag=f"w1_{i}", name=f"h3ab_{i}") for i in range(2)]
                for half in range(2):
                    nc.vector.tensor_copy(out=h3b[half][:], in_=h3[half][:])
                    nc.vector.tensor_scalar_mul(
                        out=h3ab[half][:], in0=h3[half][:], scalar1=ALPHA
                    )
                    o = 0 if half == 0 else hsplit
                    nc.sync.dma_start(out=d_hb[:, o : o + h3_sizes[half]], in_=h3b[half][:])
                    nc.sync.dma_start(out=d_hab[:, o : o + h3_sizes[half]], in_=h3ab[half][:])

            # ------------- persistent + propagation pools (post-MLP)
            with (
                tc.tile_pool(name="persist", bufs=1) as pers,
                tc.tile_pool(name="prop", bufs=1) as pr,
                tc.tile_pool(name="gbuf", bufs=2) as gb,
                tc.tile_pool(name="qbuf", bufs=2) as qb,
                tc.tile_pool(name="wps", bufs=2, space="PSUM") as wps,
                tc.tile_pool(name="mps", bufs=2, space="PSUM") as mps,
            ):
                zpk = pers.tile([128, 2 * npcp], dt.uint32)
                zpk3 = zpk[:].rearrange("p (n i) -> p n i", i=2)
                h_a = pers.tile([128, oct_, 4], dt.bfloat16)
                znew = pers.tile([128, 2 * oct_], dt.uint32)
                idx_t = pers.tile([128, idx_cols], dt.int16)
                bidx_t = pers.tile([128, bidx_cols], dt.int16)
                sel8 = pers.tile([NG, 128], dt.float32)
                mrg = pers.tile([128, 128], dt.float32)

                nc.sync.dma_start(out=idx_t[:], in_=d_idx[:])
                nc.sync.dma_start(out=bidx_t[:], in_=d_bidx[:])
                nc.sync.dma_start(out=sel8[:], in_=d_sel8[:])
                nc.sync.dma_start(out=mrg[:], in_=d_merge[:])

                # ---- pack h into h_a / znew (z0 = h) via octant staging +
                # one strided DVE copy per plane (avoids per-element DMA).
                znew_v = znew[:].bitcast(dt.bfloat16).rearrange(
                    "p (n j) -> p n j", j=4
                )  # [128, oct, 4]
                msg = pr.tile([128, 4 * PMAX], dt.bfloat16)
                pref = pr.tile([128, PMAX, 4], dt.float32)
                pref_bf = pref[:].rearrange("p n j -> p (n j)").bitcast(dt.bfloat16)
                for j in range(4):
                    sg_b = msg[:, :oct_]
                    sg_a = pref_bf[:, :oct_]
                    for o in range(NG):
                        lo = o * oct_
                        w_ = min(oct_, npc - lo)
                        if w_ <= 0:
                            continue
                        nc.sync.dma_start(
                            out=sg_b[16 * o : 16 * o + 16, :w_],
                            in_=d_hb[16 * j : 16 * j + 16, lo : lo + w_],
                        )
                        nc.sync.dma_start(
                            out=sg_a[16 * o : 16 * o + 16, :w_],
                            in_=d_hab[16 * j : 16 * j + 16, lo : lo + w_],
                        )
                    nc.vector.tensor_copy(out=znew_v[:, :, j], in_=sg_b)
                    nc.vector.tensor_copy(out=h_a[:, :, j], in_=sg_a)

                s_sb = pr.tile([128, 4 * NBIMAX], dt.float32)
                stg = pr.tile([128, 4 * NBIMAX], dt.float32)
                zst = pr.tile([128, 4 * NBIMAX], dt.bfloat16, bufs=2)

                def do_allgather(it):
                    for o in range(NG):
                        nc.sync.dma_start(
                            out=cc_in[:, 4 * oct_ * o : 4 * oct_ * (o + 1)],
                            in_=znew[16 * o : 16 * o + 16, :].bitcast(dt.bfloat16),
                        )
                    nc.gpsimd.collective_compute(
                        "AllGather",
                        AOP.bypass,
                        replica_groups=[list(range(M))],
                        ins=[cc_in[:]],
                        outs=[cc_out[:]],
                    )
                    nc.sync.dma_start(
                        out=zpk[:].bitcast(dt.bfloat16), in_=cc_out[:]
                    )

                do_allgather(-1)  # distribute z0 = h

                for it in range(k_iters):
                    last = it == k_iters - 1
                    for ri, r in enumerate(ranges):
                        p, nr, n0, nbi = r["p"], r["nr"], r["n0"], r["nbi"]
                        o = n0 // oct_
                        n0l = n0 - o * oct_
                        rows = slice(16 * o, 16 * o + 16)
                        g_t = gb.tile([128, 2 * PMAX], dt.uint32, tag="g", bufs=3)
                        nc.gpsimd.ap_gather(
                            out_ap=g_t[:, : 2 * p].rearrange(
                                "a (n i) -> a n i", i=2
                            ),
                            in_ap=zpk3,
                            idxs_ap=idx_t[:, r["io"] : r["io"] + p // 16],
                            channels=128,
                            num_elems=npcp,
                            d=2,
                            num_idxs=p,
                        )
                        # per-edge weight broadcast into psum [128, p] fp32
                        w_sb = gb.tile([NG, PMAX], dt.float32, tag="wsb")
                        nc.sync.dma_start(
                            out=w_sb[:, :p], in_=d_wf[:, r["wo"] : r["wo"] + p]
                        )
                        pw = wps.tile([128, 1024], dt.float32, space="PSUM", tag="pw")
                        for jj in range((p + 511) // 512):
                            e = min(512, p - 512 * jj)
                            nc.tensor.matmul(
                                out=pw[:, 512 * jj : 512 * jj + e],
                                lhsT=sel8[:],
                                rhs=w_sb[:, 512 * jj : 512 * jj + e],
                                start=True,
                                stop=True,
                            )
                        # mult: de-interleave 4 planes, bf16 msg
                        gvt = g_t[:, : 2 * p].bitcast(dt.bfloat16).rearrange(
                            "a (n j) -> a j n", j=4
                        )
                        # interleave mult/scan per plane so the last scan
                        # (which gates the next main gather via the tile-
                        # inserted semaphore) completes as early as possible
                        for j in range(4):
                            nc.vector.tensor_tensor(
                                out=msg[:, j * PMAX : j * PMAX + p],
                                in0=gvt[:, j, :],
                                in1=pw[:, :p],
                                op=AOP.mult,
                            )
                            nc.vector.tensor_tensor_scan(
                                out=pref[:, :p, j],
                                data0=msg[:, j * PMAX : j * PMAX + p],
                                data1=msg[:, j * PMAX : j * PMAX + p],
                                initial=0.0,
                                op0=AOP.add,
                                op1=AOP.bypass,
                            )
                        # segment-end gather (d=4: all 4 planes per node)
                        q_t = qb.tile([128, NBIMAX, 4], dt.float32, tag="q")
                        nc.gpsimd.ap_gather(
                            out_ap=q_t[:, :nbi, :],
                            in_ap=pref[:, :p, :],
                            idxs_ap=bidx_t[:, r["bo"] : r["bo"] + r["nbi_cols"]],
                            channels=128,
                            num_elems=p,
                            d=4,
                            num_idxs=nbi,
                        )
                        # merge 8 group partials; octant rows get the sum
                        pm = mps.tile([128, 1024], dt.float32, space="PSUM", tag="pm")
                        q_flat = q_t[:, :nbi, :].rearrange("a n j -> a (n j)")
                        for jj in range((4 * nbi + 511) // 512):
                            e = min(512, 4 * nbi - 512 * jj)
                            nc.tensor.matmul(
                                out=pm[:, 512 * jj : 512 * jj + e],
                                lhsT=mrg[:],
                                rhs=q_flat[:, 512 * jj : 512 * jj + e],
                                start=True,
                                stop=True,
                            )
                        # All DVE ops run on full 128 rows (partition base
                        # must be 32-aligned): merge output is replicated
                        # mod-16 across rows, and non-octant rows compute
                        # junk that the 16-row DMA below never reads.
                        nc.scalar.activation(
                            out=s_sb[:, : 4 * nr],
                            in_=pm[:, : 4 * nr],
                            func=mybir.ActivationFunctionType.Copy,
                            scale=1.0,
                        )
                        sv = s_sb[:, : 4 * nr].rearrange("p (n j) -> p n j", j=4)
                        tv = stg[:, : 4 * nr].rearrange("p (n j) -> p n j", j=4)
                        nc.vector.tensor_copy(out=tv[:, 0:1, :], in_=sv[:, 0:1, :])
                        if nr > 1:
                            nc.vector.tensor_tensor(
                                out=tv[:, 1:nr, :],
                                in0=sv[:, 1:nr, :],
                                in1=sv[:, 0 : nr - 1, :],
                                op=AOP.subtract,
                            )
                        # epilogue: z = seg + alpha*h
                        if last:
                            nc.vector.tensor_tensor(
                                out=tv[:, :nr, :],
                                in0=tv[:, :nr, :],
                                in1=h_a[:, n0l : n0l + nr, :],
                                op=AOP.add,
                            )
                            nc.sync.dma_start(
                                out=d_zout[rows, n0l : n0l + nr, :],
                                in_=stg[rows, : 4 * nr].rearrange(
                                    "p (n j) -> p n j", j=4
                                ),
                            )
                        else:
                            zv = zst[:, : 4 * nr].rearrange("p (n j) -> p n j", j=4)
                            nc.vector.tensor_tensor(
                                out=zv,
                                in0=tv[:, :nr, :],
                                in1=h_a[:, n0l : n0l + nr, :],
                                op=AOP.add,
                            )
                            nc.sync.dma_start(
                                out=znew_v[rows, n0l : n0l + nr, :],
                                in_=zst[rows, : 4 * nr].rearrange(
                                    "p (n j) -> p n j", j=4
                                ),
                            )
                    if not last:
                        do_allgather(it)

    nc.compile()
    return nc


# ---------------------------------------------------------------- runner
def run(x, W1, b1, W2, b2, W3, b3, edge_weight, edge_src, edge_dst, k_iters=10, trace=False):
    n_nodes, in_ch = x.shape
    hid_ch = W1.shape[1]
    out_ch = W3.shape[1]
    npc = n_nodes // M
    plans = build_plan(
        n_nodes,
        np.asarray(edge_src, dtype=np.int64),
        np.asarray(edge_dst, dtype=np.int64),
        np.asarray(edge_weight, dtype=np.float32),
    )
    oct_ = plans[0]["oct"]
    nc = build_kernel(n_nodes, in_ch, hid_ch, out_ch, k_iters, plans)
    in_maps = host_inputs(
        np.asarray(x), np.asarray(W1), np.asarray(b1), np.asarray(W2),
        np.asarray(b2), np.asarray(W3), np.asarray(b3), plans, n_nodes,
    )
    res = run_bass_kernel_spmd(nc, in_maps, list(range(M)), trace=trace)
    out = np.empty((n_nodes, out_ch), dtype=np.float32)
    for c in range(M):
        zo = res.results[c]["zout"]  # [128, oct, 4]
        zc = np.empty((out_ch, npc), dtype=np.float32)
        for o in range(NG):
            lo = o * oct_
            w_ = min(oct_, npc - lo)
            if w_ <= 0:
                continue
            for l in range(16):
                for j in range(4):
                    zc[4 * l + j, lo : lo + w_] = zo[16 * o + l, :w_, j]
        out[c * npc : (c + 1) * npc] = zc.T
    return out, res


# ---------------------------------------------------------------- entry point
N_NODES = 100000
K_ITERS = 10


def kernel(**inputs):
    """Full (unsharded) inputs -> full [100000, 64] float32 output."""
    out, _ = run(
        np.asarray(inputs["x"], dtype=np.float32),
        np.asarray(inputs["W1"], dtype=np.float32),
        np.asarray(inputs["b1"], dtype=np.float32),
        np.asarray(inputs["W2"], dtype=np.float32),
        np.asarray(inputs["b2"], dtype=np.float32),
        np.asarray(inputs["W3"], dtype=np.float32),
        np.asarray(inputs["b3"], dtype=np.float32),
        np.asarray(inputs["edge_weight"], dtype=np.float32),
        np.asarray(inputs["edge_src"]),
        np.asarray(inputs["edge_dst"]),
        k_iters=K_ITERS,
        trace=False,
    )
    return out
